# revision 2
# baseline (speedup 1.0000x reference)
"""Trainium2 Bass kernel for BinaryLinear: y = x @ (aa*tanh(kk*W)).T + bias.

Sharding: data-parallel over the flattened M = B*S dimension (8 cores x 1024
rows each). Each core receives its x shard plus the full weight/bias and
computes its y rows independently -- no collectives. This sharding minimizes
HBM traffic (x read once, W read per-core) and needs no inter-core exchange.

Per-core pipeline -- the PE runs nothing but matmuls; all transposes ride the
DMA xbar and all casts ride SWDGE cast-DMAs:
  1. x shard [1024, 4096] f32: per 128-row block, SWDGE cast-DMA -> f16,
     then one HWDGE transpose-DMA [128, 4096] -> xT slab [128k, 32ko, 128m].
  2. W [4096, 4096] f32: per 128-row block, SWDGE cast-DMA -> f16, ACT
     tanh(kk*w) (f16 in/out), HWDGE transpose-DMA into the double-buffered
     wbT slab [128k, 32ko, 512o].
  3. Per (o-tile, m-tile): 32 chained [128x128x512] f16 matmuls into one
     PSUM bank; DVE computes aa*psum + bias; store from SBUF.
"""

import numpy as np

B, S, DIN, DOUT = 4, 2048, 4096, 4096
N_CORES = 8
M_TOTAL = B * S
M_SHARD = M_TOTAL // N_CORES
P = 128


def build_nc(m_shard=M_SHARD, din=DIN, dout=DOUT, o_tile=512,
             n_swq=2, slab_bufs=2, stg_bufs=2, psum_bufs=8, out_bufs=4,
             repeat=None):
    import concourse.bass as bass
    import concourse.mybir as mybir
    import concourse.tile as tile
    from concourse import bacc
    from contextlib import ExitStack

    f32 = mybir.dt.float32
    f16 = mybir.dt.float16

    assert m_shard % P == 0 and din % P == 0
    assert dout % o_tile == 0 and o_tile % P == 0 and o_tile <= 512

    KO = din // P          # k-tiles of 128
    MT = m_shard // P      # m-tiles of 128
    OT = dout // o_tile    # o-tiles
    OP = o_tile // P       # 128-row weight blocks per o-tile

    nc = bacc.Bacc("TRN2", target_bir_lowering=False, debug=False,
                   num_devices=N_CORES, num_swdge_queues=n_swq)

    x_d = nc.dram_tensor("x", [m_shard, din], f32, kind="ExternalInput").ap()
    w_d = nc.dram_tensor("weight", [dout, din], f32, kind="ExternalInput").ap()
    b_d = nc.dram_tensor("bias", [1, dout], f32, kind="ExternalInput").ap()
    kk_d = nc.dram_tensor("kk", [1, 1], f32, kind="ExternalInput").ap()
    aa_d = nc.dram_tensor("aa", [1, 1], f32, kind="ExternalInput").ap()
    y_d = nc.dram_tensor("y", [m_shard, dout], f32, kind="ExternalOutput").ap()

    with tile.TileContext(nc) as tc, ExitStack() as ctx:
        singles = ctx.enter_context(tc.tile_pool(name="singles", bufs=1))
        x_stage = ctx.enter_context(tc.tile_pool(name="xstg", bufs=stg_bufs))
        w_stage = ctx.enter_context(tc.tile_pool(name="wstg", bufs=stg_bufs))
        wb_stage = ctx.enter_context(tc.tile_pool(name="wbstg", bufs=stg_bufs))
        xt_pool = ctx.enter_context(tc.tile_pool(name="xt", bufs=1))
        w_pool = ctx.enter_context(tc.tile_pool(name="wslab", bufs=slab_bufs))
        out_pool = ctx.enter_context(tc.tile_pool(name="outp", bufs=out_bufs))
        psum_pool = ctx.enter_context(
            tc.tile_pool(name="psum", bufs=psum_bufs, space="PSUM"))

        # Runtime scalars kk/aa broadcast to one value per partition.
        scal = singles.tile([P, 2], f32)
        nc.gpsimd.dma_start(out=scal[:, 0:1], in_=kk_d.to_broadcast([P, 1]))
        nc.gpsimd.dma_start(out=scal[:, 1:2], in_=aa_d.to_broadcast([P, 1]))
        kk_ap = scal[:, 0:1]
        aa_ap = scal[:, 1:2]

        # Bias replicated across partitions (free-dim add at evacuation).
        bias_rep = singles.tile([P, dout], f32)
        nc.gpsimd.dma_start(out=bias_rep, in_=b_d.to_broadcast([P, dout]))

        def body():
            xT = xt_pool.tile([P, KO, m_shard], f16)

            def prep_x(mt):
                x16 = x_stage.tile([P, din], f16, tag="xstg")
                nc.gpsimd.dma_start(
                    out=x16, in_=x_d[mt * P:(mt + 1) * P, :])
                nc.sync.dma_start(
                    out=xT[:, :, mt * P:(mt + 1) * P], in_=x16,
                    transpose=True)

            def produce_slab(ot):
                slab = w_pool.tile([P, KO, o_tile], f16, tag="slab")
                for op in range(OP):
                    row0 = ot * o_tile + op * P
                    w16 = w_stage.tile([P, din], f16, tag="wstg")
                    nc.gpsimd.dma_start(out=w16, in_=w_d[row0:row0 + P, :])
                    wb = wb_stage.tile([P, din], f16, tag="wbstg")
                    nc.scalar.activation(
                        wb, w16, mybir.ActivationFunctionType.Tanh,
                        scale=kk_ap)
                    nc.sync.dma_start(
                        out=slab[:, :, op * P:(op + 1) * P], in_=wb,
                        transpose=True)
                return slab

            prep_x(0)
            slabs = [produce_slab(0)]
            for mt in range(1, MT):
                prep_x(mt)
            if OT > 1:
                slabs.append(produce_slab(1))

            for ot in range(OT):
                slab = slabs[ot]
                for mt in range(MT):
                    ps = psum_pool.tile([P, o_tile], f32, tag="mmps")
                    for ko in range(KO):
                        nc.tensor.matmul(
                            ps,
                            lhsT=xT[:, ko, mt * P:(mt + 1) * P],
                            rhs=slab[:, ko, :],
                            start=(ko == 0),
                            stop=(ko == KO - 1))
                    ob = out_pool.tile([P, o_tile], f32)
                    nc.vector.scalar_tensor_tensor(
                        out=ob, in0=ps, scalar=aa_ap,
                        in1=bias_rep[:, ot * o_tile:(ot + 1) * o_tile],
                        op0=mybir.AluOpType.mult,
                        op1=mybir.AluOpType.add)
                    nc.scalar.dma_start(
                        out=y_d[mt * P:(mt + 1) * P,
                                ot * o_tile:(ot + 1) * o_tile],
                        in_=ob)
                if ot + 2 < OT:
                    slabs.append(produce_slab(ot + 2))

        if repeat is None:
            body()
        else:
            with tc.For_i(0, repeat, 1):
                body()

    nc.compile()
    return nc


def make_in_maps(x, weight, bias, kk, aa, n_cores=N_CORES, m_shard=None):
    x = np.ascontiguousarray(np.asarray(x, dtype=np.float32))
    m_total = x.size // x.shape[-1]
    din = x.shape[-1]
    if m_shard is None:
        m_shard = m_total // n_cores
    xf = x.reshape(m_total, din)
    w = np.ascontiguousarray(np.asarray(weight, dtype=np.float32))
    b = np.ascontiguousarray(
        np.asarray(bias, dtype=np.float32).reshape(1, -1))
    kk2 = np.asarray(kk, dtype=np.float32).reshape(1, 1).copy()
    aa2 = np.asarray(aa, dtype=np.float32).reshape(1, 1).copy()
    return [
        {
            "x": np.ascontiguousarray(xf[c * m_shard:(c + 1) * m_shard]),
            "weight": w,
            "bias": b,
            "kk": kk2,
            "aa": aa2,
        }
        for c in range(n_cores)
    ]


def run_on_cores(nc, in_maps, trace=False, **kwargs):
    from concourse.bass_utils import run_bass_kernel_spmd
    return run_bass_kernel_spmd(nc, in_maps,
                                core_ids=list(range(len(in_maps))),
                                trace=trace, **kwargs)


_NC_CACHE = None


def kernel(**inputs):
    global _NC_CACHE
    if _NC_CACHE is None:
        _NC_CACHE = build_nc()
    nc = _NC_CACHE
    in_maps = make_in_maps(inputs["x"], inputs["weight"], inputs["bias"],
                           inputs["kk"], inputs["aa"])
    res = run_on_cores(nc, in_maps, trace=False)
    y = np.concatenate([r["y"] for r in res.results], axis=0)
    return y.reshape(B, S, DOUT).astype(np.float32, copy=False)
